# revision 4
# baseline (speedup 1.0000x reference)
# Multi-head attention kernel for 8 Trainium2 NeuronCores.
#
# Problem: x [4, 1536, 2048] fp32, channels = (qkv, head, dim) with h=8, d=64.
# Returns (o [4, 512, 2048], qk_softmax [4, 8, 2048, 2048]).
#
# Sharding: head-parallel. Core c computes head c for all 4 batches —
# attention is independent per (batch, head), and the dominant cost is
# writing the 537 MB softmax matrix (67 MB/core), so the kernel is
# HBM-write bound (~210 us/core roofline at ~358 GB/s).
#
# Per (b, h) pair on a core:
#   Phase A: S^T bands [128j, L i] = k_block^T @ q  (float32r matmuls)
#            -> exp(S^T / sqrt(8)) on ACT -> P^T bands (bf16, SBUF).
#   AV:      lhsT = [v^T | ones] (65 cols): PSUM rows 0..63 accumulate
#            unnormalized o^T, row 64 accumulates the softmax denominator
#            s(i) for free.  o^T normalized with r = 1/s broadcast across
#            partitions via a tiny PE outer product.
#   Phase B: row-major P tiles via PE block-transposes of P^T (bf16,
#            1 cyc/row), then one fused DVE tensor_scalar (x r[i]) does
#            PSUM->SBUF copy + normalization -> DMA to HBM.

import numpy as np

B = 4
H = 8
D = 64
L = 2048
NB = L // 128          # j-bands / i-tiles
NH = 2                 # i halves for PSUM pressure
LH = L // NH           # 1024
RQ = 512               # row-quarter width in phase B (1 PSUM bank granularity)
INV_SCALE = 1.0 / float(np.sqrt(D / H))  # 1/sqrt(8)

_CACHE = {}


def _build():
    from contextlib import ExitStack

    import concourse.bass as bass  # noqa: F401
    import concourse.mybir as mybir
    import concourse.tile as tile
    from concourse import bacc
    from concourse.masks import make_identity

    dt = mybir.dt

    nc = bacc.Bacc(
        "TRN2",
        target_bir_lowering=False,
        debug=False,
        enable_asserts=False,
        num_devices=8,
    )
    xq = nc.dram_tensor("xq", [B, D, L], dt.float32, kind="ExternalInput").ap()
    xk = nc.dram_tensor("xk", [B, D, L], dt.float32, kind="ExternalInput").ap()
    xv = nc.dram_tensor("xv", [B, D, L], dt.float32, kind="ExternalInput").ap()
    o_out = nc.dram_tensor("o", [B, D, L], dt.float32, kind="ExternalOutput").ap()
    p_out = nc.dram_tensor("p", [B, L, L], dt.float32, kind="ExternalOutput").ap()

    with tile.TileContext(nc) as tc, ExitStack() as ctx:
        const = ctx.enter_context(tc.tile_pool(name="const", bufs=1))
        qkv_pool = ctx.enter_context(tc.tile_pool(name="qkv", bufs=2))
        pt_pool = ctx.enter_context(tc.tile_pool(name="pt", bufs=NB + 2))
        vt_pool = ctx.enter_context(tc.tile_pool(name="vt", bufs=2))
        small = ctx.enter_context(tc.tile_pool(name="small", bufs=4))
        rowsb = ctx.enter_context(tc.tile_pool(name="rowsb", bufs=6))
        osb_pool = ctx.enter_context(tc.tile_pool(name="osb", bufs=3))
        psum_st = ctx.enter_context(tc.tile_pool(name="pst", bufs=1, space="PSUM"))
        psum_o = ctx.enter_context(tc.tile_pool(name="po", bufs=2, space="PSUM"))
        psum_row = ctx.enter_context(tc.tile_pool(name="prow", bufs=2, space="PSUM"))

        id128b = const.tile([128, 128], dt.bfloat16)
        make_identity(nc, id128b)
        id64f = const.tile([64, 64], dt.float32)
        make_identity(nc, id64f)
        ones64 = const.tile([1, 64], dt.float32)
        nc.vector.memset(ones64, 1.0)
        ones11 = const.tile([1, 1], dt.float32)
        nc.vector.memset(ones11, 1.0)

        for b in range(B):
            q = qkv_pool.tile([D, L], dt.float32r, tag="q")
            k = qkv_pool.tile([D, L], dt.float32r, tag="k")
            v = qkv_pool.tile([D, L], dt.float32, tag="v")
            nc.sync.dma_start(q[:], xq[b].bitcast(dt.float32r))
            nc.sync.dma_start(k[:], xk[b].bitcast(dt.float32r))
            nc.sync.dma_start(v[:], xv[b])
            qr = q
            kr = k

            # v^T (bf16) with an appended ones column: [128, NB, 66]
            # (col 64 = 1.0 -> AV accumulates the softmax denominator in
            # PSUM row 64; col 65 is padding).
            vt1 = vt_pool.tile([128, NB, 66], dt.bfloat16, tag="vt1")
            nc.vector.memset(vt1[:, :, 64:65], 1.0)
            vt_ps = psum_st.tile([128, LH], dt.float32, tag="st")
            for jb in range(NB):
                nc.tensor.transpose(
                    vt_ps[:, jb * 64 : (jb + 1) * 64],
                    v[:, jb * 128 : (jb + 1) * 128],
                    id64f,
                )
            nc.scalar.activation(
                vt1[:, :, 0:64],
                vt_ps.rearrange("p (n d) -> p n d", n=NB),
                mybir.ActivationFunctionType.Copy,
            )

            # Phase A: S^T = k^T q in [128, LH] half-bands, exp -> P^T bf16.
            pt = []
            for jb in range(NB):
                ptb = pt_pool.tile([128, L], dt.bfloat16, tag="pt")
                pt.append(ptb)
                for ih in range(NH):
                    st = psum_st.tile([128, LH], dt.float32, tag="st")
                    for n in range(LH // 512):
                        nc.tensor.matmul(
                            st[:, n * 512 : (n + 1) * 512],
                            lhsT=kr[:, jb * 128 : (jb + 1) * 128],
                            rhs=qr[:, ih * LH + n * 512 : ih * LH + (n + 1) * 512],
                            start=True,
                            stop=True,
                        )
                    for n in range(LH // 512):
                        nc.scalar.activation(
                            ptb[:, ih * LH + n * 512 : ih * LH + (n + 1) * 512],
                            st[:, n * 512 : (n + 1) * 512],
                            mybir.ActivationFunctionType.Exp,
                            scale=INV_SCALE,
                        )

            # AV + denominator per i-half, then normalize o^T.
            r_all = small.tile([128, NB], dt.float32, tag="rall")
            for ih in range(NH):
                ov = psum_o.tile([128, LH], dt.float32, tag="o")
                for jb in range(NB):
                    for n in range(LH // 512):
                        nc.tensor.matmul(
                            ov[0:65, n * 512 : (n + 1) * 512],
                            lhsT=vt1[:, jb, 0:65],
                            rhs=pt[jb][:, ih * LH + n * 512 : ih * LH + (n + 1) * 512],
                            start=(jb == 0),
                            stop=(jb == NB - 1),
                        )
                rt = small.tile([1, LH], dt.float32, tag="rt")
                nc.vector.reciprocal(rt[:], ov[64:65, :])
                # r broadcast to 64 partitions: outer(ones64, r).
                r64 = psum_o.tile([128, LH], dt.float32, tag="o")
                for n in range(LH // 512):
                    nc.tensor.matmul(
                        r64[0:64, n * 512 : (n + 1) * 512],
                        lhsT=ones64[:],
                        rhs=rt[:, n * 512 : (n + 1) * 512],
                        start=True,
                        stop=True,
                    )
                r64sb = osb_pool.tile([D, LH], dt.float32, tag="r64sb")
                nc.scalar.copy(r64sb[:], r64[0:64, :])
                osb = osb_pool.tile([D, LH], dt.float32, tag="osb")
                nc.vector.tensor_mul(osb[:], ov[0:64, :], r64sb[:])
                nc.sync.dma_start(o_out[b][:, ih * LH : (ih + 1) * LH], osb[:])
                # r as [128, 1] columns per i-tile (transpose via K=1 matmuls).
                rc = psum_st.tile([128, LH], dt.float32, tag="st")
                for t in range(LH // 128):
                    nc.tensor.matmul(
                        rc[:, t : t + 1],
                        lhsT=rt[:, t * 128 : (t + 1) * 128],
                        rhs=ones11[:],
                        start=True,
                        stop=True,
                    )
                nc.vector.tensor_copy(
                    r_all[:, ih * (LH // 128) : (ih + 1) * (LH // 128)],
                    rc[:, 0 : LH // 128],
                )

            # Phase B: row-major normalized P via PE transposes + fused
            # DVE (PSUM->SBUF copy * r) -> DMA.
            for it in range(NB):
                for rq in range(L // RQ):
                    rp = psum_row.tile([128, RQ], dt.bfloat16, tag="row")
                    for u in range(RQ // 128):
                        jb = rq * (RQ // 128) + u
                        nc.tensor.transpose(
                            rp[:, u * 128 : (u + 1) * 128],
                            pt[jb][:, it * 128 : (it + 1) * 128],
                            id128b,
                        )
                    rsb = rowsb.tile([128, RQ], dt.float32, tag="rsb")
                    nc.vector.tensor_scalar(
                        out=rsb[:],
                        in0=rp[:],
                        scalar1=r_all[:, it : it + 1],
                        scalar2=None,
                        op0=mybir.AluOpType.mult,
                    )
                    nc.sync.dma_start(
                        p_out[b, it * 128 : (it + 1) * 128, rq * RQ : (rq + 1) * RQ],
                        rsb[:],
                    )

    nc.compile()
    return nc


def _get_nc():
    if "nc" not in _CACHE:
        _CACHE["nc"] = _build()
    return _CACHE["nc"]


def kernel(x: np.ndarray):
    from concourse.bass_utils import run_bass_kernel_spmd

    x = np.ascontiguousarray(np.asarray(x, dtype=np.float32))
    assert x.shape == (B, 3 * H * D, L), x.shape

    nc = _get_nc()
    in_maps = []
    for c in range(H):
        in_maps.append(
            {
                "xq": np.ascontiguousarray(x[:, c * D : (c + 1) * D, :]),
                "xk": np.ascontiguousarray(x[:, H * D + c * D : H * D + (c + 1) * D, :]),
                "xv": np.ascontiguousarray(
                    x[:, 2 * H * D + c * D : 2 * H * D + (c + 1) * D, :]
                ),
            }
        )
    res = run_bass_kernel_spmd(nc, in_maps, core_ids=list(range(H)))
    _CACHE["last_result"] = res

    o_full = np.empty((B, H * D, L), dtype=np.float32)
    qk_full = np.empty((B, H, L, L), dtype=np.float32)
    for c in range(H):
        o_full[:, c * D : (c + 1) * D, :] = res.results[c]["o"]
        qk_full[:, c, :, :] = res.results[c]["p"]
    return (o_full, qk_full)


# revision 15
# speedup vs baseline: 1.2886x; 1.2886x over previous
# Multi-head attention kernel for 8 Trainium2 NeuronCores.
#
# Problem: x [4, 1536, 2048] fp32, channels = (qkv, head, dim) with h=8, d=64.
# Returns (o [4, 512, 2048], qk_softmax [4, 8, 2048, 2048]).
#
# Sharding: head-parallel. Core c computes head c for all 4 batches —
# attention is independent per (batch, head), and the dominant cost is
# writing the 537 MB softmax matrix (67 MB/core), so the kernel is
# HBM-write bound (~210 us/core roofline at ~358 GB/s).
#
# Per (b, h) pair on a core:
#   Phase A: S^T bands [128j, L i] = k_block^T @ q  (float32r matmuls)
#            -> exp(S^T / sqrt(8)) on ACT -> P^T bands (bf16, SBUF).
#   AV:      lhsT = [v^T | ones] (65 cols): PSUM rows 0..63 accumulate
#            unnormalized o^T, row 64 accumulates the softmax denominator
#            s(i) for free.  o^T normalized with r = 1/s broadcast across
#            partitions via a tiny PE outer product.
#   Phase B: row-major P tiles via PE block-transposes of P^T (bf16,
#            1 cyc/row), then one fused DVE tensor_scalar (x r[i]) does
#            PSUM->SBUF copy + normalization -> DMA to HBM.

import numpy as np

B = 4
H = 8
D = 64
L = 2048
NB = L // 128          # j-bands / i-tiles
NH = 2                 # i halves for PSUM pressure
LH = L // NH           # 1024
RQ = 512               # row-quarter width in phase B (1 PSUM bank granularity)
INV_SCALE = 1.0 / float(np.sqrt(D / H))  # 1/sqrt(8)

_CACHE = {}


def _build():
    from contextlib import ExitStack

    import concourse.bass as bass  # noqa: F401
    import concourse.mybir as mybir
    import concourse.tile as tile
    from concourse import bacc
    from concourse.masks import make_identity

    dt = mybir.dt

    nc = bacc.Bacc(
        "TRN2",
        target_bir_lowering=False,
        debug=False,
        enable_asserts=False,
        num_devices=8,
    )
    xq = nc.dram_tensor("xq", [B, D, L], dt.float32, kind="ExternalInput").ap()
    xk = nc.dram_tensor("xk", [B, D, L], dt.float32, kind="ExternalInput").ap()
    xv = nc.dram_tensor("xv", [B, D, L], dt.float32, kind="ExternalInput").ap()
    o_out = nc.dram_tensor("o", [B, D, L], dt.float32, kind="ExternalOutput").ap()
    p_out = nc.dram_tensor("p", [B, L, L], dt.float32, kind="ExternalOutput").ap()

    with tile.TileContext(nc) as tc, ExitStack() as ctx:
        const = ctx.enter_context(tc.tile_pool(name="const", bufs=1))
        qkv_pool = ctx.enter_context(tc.tile_pool(name="qkv", bufs=2))
        pt_pool = ctx.enter_context(tc.tile_pool(name="pt", bufs=NB + 2))
        vt_pool = ctx.enter_context(tc.tile_pool(name="vt", bufs=2))
        small = ctx.enter_context(tc.tile_pool(name="small", bufs=3))
        rowsb = ctx.enter_context(tc.tile_pool(name="rowsb", bufs=10))
        osb_pool = ctx.enter_context(tc.tile_pool(name="osb", bufs=2))
        psum_st = ctx.enter_context(tc.tile_pool(name="pst", bufs=1, space="PSUM"))
        psum_o = ctx.enter_context(tc.tile_pool(name="po", bufs=2, space="PSUM"))
        psum_row = ctx.enter_context(tc.tile_pool(name="prow", bufs=2, space="PSUM"))

        id64f = const.tile([64, 64], dt.float32)
        make_identity(nc, id64f)
        ones64 = const.tile([1, 64], dt.float32)
        nc.vector.memset(ones64, 1.0)
        ones11 = const.tile([1, 1], dt.float32)
        nc.vector.memset(ones11, 1.0)

        for b in range(B):
            q = qkv_pool.tile([D, L], dt.float32r, tag="q")
            k = qkv_pool.tile([D, L], dt.float32r, tag="k")
            v = qkv_pool.tile([D, L], dt.float32, tag="v")
            nc.sync.dma_start(q[:], xq[b].bitcast(dt.float32r))
            nc.sync.dma_start(k[:], xk[b].bitcast(dt.float32r))
            nc.sync.dma_start(v[:], xv[b])
            qr = q
            kr = k
            # bf16 copies feed the S^T pass (fp32r matmul measured ~1.5x
            # slower than bf16; the transposed pass only feeds o and the
            # softmax denominator, where bf16 noise averages out).
            qb = qkv_pool.tile([D, L], dt.bfloat16, tag="qb")
            kb = qkv_pool.tile([D, L], dt.bfloat16, tag="kb")
            nc.vector.tensor_copy(qb[:], q.bitcast(dt.float32))
            nc.vector.tensor_copy(kb[:], k.bitcast(dt.float32))

            # v^T (bf16) with an appended ones column: [128, NB, 66]
            # (col 64 = 1.0 -> AV accumulates the softmax denominator in
            # PSUM row 64; col 65 is padding).
            vt1 = vt_pool.tile([128, NB, 66], dt.bfloat16, tag="vt1")
            nc.vector.memset(vt1[:, :, 64:65], 1.0)
            vt_ps = psum_st.tile([128, LH], dt.float32, tag="st")
            for jb in range(NB):
                nc.tensor.transpose(
                    vt_ps[:, jb * 64 : (jb + 1) * 64],
                    v[:, jb * 128 : (jb + 1) * 128],
                    id64f,
                )
            nc.vector.tensor_copy(
                vt1[:, :, 0:64],
                vt_ps.rearrange("p (n d) -> p n d", n=NB),
            )

            # Phase A: S^T = k^T q in [128, LH] half-bands, exp -> P^T bf16.
            pt = []
            for jb in range(NB):
                ptb = pt_pool.tile([128, L], dt.bfloat16, tag="pt")
                pt.append(ptb)
                for ih in range(NH):
                    st = psum_st.tile([128, LH], dt.float32, tag="st")
                    for n in range(LH // 512):
                        nc.tensor.matmul(
                            st[:, n * 512 : (n + 1) * 512],
                            lhsT=kb[:, jb * 128 : (jb + 1) * 128],
                            rhs=qb[:, ih * LH + n * 512 : ih * LH + (n + 1) * 512],
                            start=True,
                            stop=True,
                        )
                    for n in range(LH // 512):
                        nc.scalar.activation(
                            ptb[:, ih * LH + n * 512 : ih * LH + (n + 1) * 512],
                            st[:, n * 512 : (n + 1) * 512],
                            mybir.ActivationFunctionType.Exp,
                            scale=INV_SCALE,
                        )

            # AV + denominator per i-half, then normalize o^T.
            for ih in range(NH):
                ov = psum_o.tile([128, LH], dt.float32, tag="o")
                for jb in range(NB):
                    for n in range(LH // 512):
                        nc.tensor.matmul(
                            ov[0:65, n * 512 : (n + 1) * 512],
                            lhsT=vt1[:, jb, 0:65],
                            rhs=pt[jb][:, ih * LH + n * 512 : ih * LH + (n + 1) * 512],
                            start=(jb == 0),
                            stop=(jb == NB - 1),
                        )
                rt = small.tile([1, LH], dt.float32, tag="rt")
                nc.vector.reciprocal(rt[:], ov[64:65, :])
                # r broadcast to 64 partitions: outer(ones64, r).
                r64 = psum_o.tile([128, LH], dt.float32, tag="o")
                for n in range(LH // 512):
                    nc.tensor.matmul(
                        r64[0:64, n * 512 : (n + 1) * 512],
                        lhsT=ones64[:],
                        rhs=rt[:, n * 512 : (n + 1) * 512],
                        start=True,
                        stop=True,
                    )
                r64sb = osb_pool.tile([D, LH], dt.float32, tag="r64sb")
                nc.vector.tensor_copy(r64sb[:], r64[0:64, :])
                osb = osb_pool.tile([D, LH], dt.float32, tag="osb")
                nc.vector.tensor_mul(osb[:], ov[0:64, :], r64sb[:])
                nc.sync.dma_start(o_out[b][:, ih * LH : (ih + 1) * LH], osb[:])

            # Phase B: row-major S tiles (fp32r) -> exp with ACT accum_out
            # collecting the row sum per quarter (exact self-consistent
            # normalization; folding -ln s into the exp bias pushes
            # arguments below fp32 underflow where the Exp table returns
            # garbage) -> DVE normalize in place -> DMA.
            for it in range(NB):
                s_acc = small.tile([128, L // RQ], dt.float32, tag="sacc")
                r_row = small.tile([128, 1], dt.float32, tag="rrow")
                rsbs = []
                for n in range(L // RQ):
                    srow = psum_row.tile([128, RQ], dt.float32, tag="row")
                    nc.tensor.matmul(
                        srow[:],
                        lhsT=qr[:, it * 128 : (it + 1) * 128],
                        rhs=kr[:, n * RQ : (n + 1) * RQ],
                        start=True,
                        stop=True,
                    )
                    rsb = rowsb.tile([128, RQ], dt.float32, tag="rsb")
                    nc.scalar.activation(
                        rsb[:],
                        srow[:],
                        mybir.ActivationFunctionType.Exp,
                        scale=INV_SCALE,
                        accum_out=s_acc[:, n : n + 1],
                    )
                    rsbs.append(rsb)
                nc.vector.reduce_sum(
                    out=r_row[:], in_=s_acc[:], axis=mybir.AxisListType.X
                )
                nc.vector.reciprocal(r_row[:], r_row[:])
                for n in range(L // RQ):
                    nc.vector.tensor_scalar(
                        out=rsbs[n][:],
                        in0=rsbs[n][:],
                        scalar1=r_row[:],
                        scalar2=None,
                        op0=mybir.AluOpType.mult,
                    )
                    nc.sync.dma_start(
                        p_out[b, it * 128 : (it + 1) * 128, n * RQ : (n + 1) * RQ],
                        rsbs[n][:],
                    )

    nc.compile()
    return nc


def _get_nc():
    if "nc" not in _CACHE:
        _CACHE["nc"] = _build()
    return _CACHE["nc"]


def kernel(x: np.ndarray):
    from concourse.bass_utils import run_bass_kernel_spmd

    x = np.ascontiguousarray(np.asarray(x, dtype=np.float32))
    assert x.shape == (B, 3 * H * D, L), x.shape

    nc = _get_nc()
    in_maps = []
    for c in range(H):
        in_maps.append(
            {
                "xq": np.ascontiguousarray(x[:, c * D : (c + 1) * D, :]),
                "xk": np.ascontiguousarray(x[:, H * D + c * D : H * D + (c + 1) * D, :]),
                "xv": np.ascontiguousarray(
                    x[:, 2 * H * D + c * D : 2 * H * D + (c + 1) * D, :]
                ),
            }
        )
    res = run_bass_kernel_spmd(nc, in_maps, core_ids=list(range(H)))
    _CACHE["last_result"] = res

    o_full = np.empty((B, H * D, L), dtype=np.float32)
    qk_full = np.empty((B, H, L, L), dtype=np.float32)
    for c in range(H):
        o_full[:, c * D : (c + 1) * D, :] = res.results[c]["o"]
        qk_full[:, c, :, :] = res.results[c]["p"]
    return (o_full, qk_full)
